# revision 1
# baseline (speedup 1.0000x reference)
"""Trainium2 Bass kernel for nn_CurlyWrapperWithMetricsCFD (retrieval_knn).

Data-parallel over the query batch B=2048 across 8 NeuronCores (256 queries
per core). The x/v banks (2x50000x3) and MLP weights are replicated.

Per-core algorithm (all fp32):
  MLP      : 3-layer MLP on transposed activations (PE + ACT relu).
  Pass A   : gT[j, q] = -d2 via K=5 augmented matmuls, bank-major tiles
             [128 bank, 256 q]; each tile is stored to a DRAM scratch AND
             PE-transposed to query-major; per-512-chunk top-8 (DVE max8)
             -> candidate buffer [128, 1600]; 13 rounds max8+match_replace
             -> exact top-100; h2 = 100th smallest d2; tie-count via a
             fused compare+accumulate.
  Pass B   : gT streamed back from the DRAM scratch (bitwise-identical to
             the values selection saw, no PE recompute); mask = [g >= -h2];
             w = exp(g/(2 h2)) * mask; accumulate u = sum_j w * [v_j, 1]
             on the PE (W^T block stationary, V' moving).
  Metrics  : u_t = u[:3]/(u[3]+1e-12); cos_dist; l2 -> out [256, 6].
Host       : rows whose rank-100 boundary is an exact-fp32 tie (count != 100)
             or a near-tie (gap < 1e-5, covering PE-vs-CPU fp32 divergence)
             are recomputed exactly with jax-CPU replicating the reference
             (handles top_k index-order tie-breaking). Typically ~66/2048.

Self-contained: hardcodes all shapes for B=2048, N=50000, D=3, H=512, k=100.
"""

import os
import numpy as np

# ---------------------------------------------------------------------------
# problem constants (hardcoded per spec)
B = 2048
N = 50000
D = 3
H = 512
KNN = 100
NCORES = 8
BS = B // NCORES            # 256 queries per core
M = 2 * N                   # 100000 bank points
MP = 102400                 # padded bank (4 bands x 25600)
CBAND = MP // 4             # 25600 columns per partition band
TILW = 512                  # pass-A tile width
NTILES = MP // TILW         # 200
NCHUNK = MP // 128          # 800 pass-B chunks
TPB = CBAND // TILW         # 50 pass-A tiles per band
CPB = CBAND // 128          # 200 pass-B chunks per band
NCAND = NTILES * 8          # 1600 candidates per query
ROUNDS = 13                 # 13*8 = 104 >= 100
NEGBIG = -3.0e38            # match_replace fill
PADC = 1000.0               # pad coordinate -> g ~ -3e6, never selected
EPS_KNN = 1e-12
EPS_COS = 1e-8
GAP_THRESH = 1e-5           # host-fix near-ties at the rank-100 boundary

_prog_cache = {}


def _build_program():
    import concourse.bass as bass
    import concourse.bacc as bacc
    import concourse.mybir as mybir
    from concourse import tile

    f32 = mybir.dt.float32
    OP = mybir.AluOpType
    ACTF = mybir.ActivationFunctionType

    nc = bacc.Bacc("TRN2", target_bir_lowering=False, debug=False,
                   num_devices=NCORES)

    # ---- dram parameters -------------------------------------------------
    bankp_d = nc.declare_dram_parameter("bankp", [128, CBAND], f32, isOutput=False)
    vpack_d = nc.declare_dram_parameter("vpack", [128, NCHUNK * 4], f32, isOutput=False)
    qrep_d = nc.declare_dram_parameter("qrep", [128, BS], f32, isOutput=False)
    xint_d = nc.declare_dram_parameter("xint", [5, BS], f32, isOutput=False)
    w1a_d = nc.declare_dram_parameter("w1aug", [5, H], f32, isOutput=False)
    w2p_d = nc.declare_dram_parameter("w2p", [128, 4 * H], f32, isOutput=False)
    b2r_d = nc.declare_dram_parameter("b2row", [1, H], f32, isOutput=False)
    w3p_d = nc.declare_dram_parameter("w3p", [128, 4 * D], f32, isOutput=False)
    b3r_d = nc.declare_dram_parameter("b3row", [1, D], f32, isOutput=False)
    iden_d = nc.declare_dram_parameter("ident", [128, 128], f32, isOutput=False)
    out_d = nc.declare_dram_parameter("out", [BS, 6], f32, isOutput=True)
    aux_d = nc.declare_dram_parameter("aux", [BS, 9], f32, isOutput=True)
    gdram_d = nc.dram_tensor("gscratch", [NCHUNK * 128, BS], f32)

    with tile.TileContext(nc) as tc:
        from contextlib import ExitStack
        with ExitStack() as ctx:
            cp = ctx.enter_context(tc.tile_pool(name="const", bufs=1))
            # ---- constant loads ------------------------------------------
            bankp = cp.tile([128, CBAND], f32)
            nc.sync.dma_start(bankp[:], bankp_d[:])
            vpack = cp.tile([128, NCHUNK * 4], f32)
            nc.sync.dma_start(vpack[:], vpack_d[:])
            qrep = cp.tile([128, BS], f32)
            nc.sync.dma_start(qrep[:], qrep_d[:])
            ident = cp.tile([128, 128], f32)
            nc.sync.dma_start(ident[:], iden_d[:])
            ones_row = cp.tile([1, BS], f32)
            nc.vector.memset(ones_row[:], 1.0)

            # persistent small tiles
            xdT = cp.tile([3, BS], f32)            # MLP output, transposed
            cand = [cp.tile([128, NCAND], f32, name=f"cand{b}", tag=f"cand{b}") for b in range(2)]
            r13 = [cp.tile([128, 8], f32, name=f"r13{b}", tag=f"r13{b}") for b in range(2)]
            cnt = [cp.tile([128, 1], f32, name=f"cnt{b}", tag=f"cnt{b}") for b in range(2)]
            junk = cp.tile([128, NCAND], f32)      # accum_out elementwise dump
            negh2_rep = cp.tile([128, BS], f32)
            s_rep = cp.tile([128, BS], f32)
            u4 = [cp.tile([128, 4], f32, name=f"u4{b}", tag=f"u4{b}") for b in range(2)]
            xdb = [cp.tile([128, 3], f32, name=f"xdb{b}", tag=f"xdb{b}") for b in range(2)]

            # ---- MLP (transposed activations) ----------------------------
            with tc.tile_pool(name="mlp", bufs=1) as mp, \
                 tc.tile_pool(name="mlpps", bufs=2, space="PSUM") as mpps:
                xint = mp.tile([5, BS], f32)
                nc.sync.dma_start(xint[:], xint_d[:])
                w1a = mp.tile([5, H], f32)
                nc.sync.dma_start(w1a[:], w1a_d[:])
                w2p = mp.tile([128, 4 * H], f32)
                nc.sync.dma_start(w2p[:], w2p_d[:])
                b2r = mp.tile([1, H], f32)
                nc.sync.dma_start(b2r[:], b2r_d[:])
                w3p = mp.tile([128, 4 * D], f32)
                nc.sync.dma_start(w3p[:], w3p_d[:])
                b3r = mp.tile([1, D], f32)
                nc.sync.dma_start(b3r[:], b3r_d[:])

                h1T = mp.tile([128, 4 * BS], f32)   # [hb*BS + q]
                for hb in range(4):
                    ps = mpps.tile([128, BS], f32, tag="mlp1")
                    nc.tensor.matmul(ps[:], w1a[:, hb * 128:(hb + 1) * 128], xint[:])
                    nc.scalar.activation(h1T[:, hb * BS:(hb + 1) * BS], ps[:], ACTF.Relu)
                h2T = mp.tile([128, 4 * BS], f32)
                for hb in range(4):
                    ps = mpps.tile([128, BS], f32, tag="mlp2")
                    for c in range(4):
                        nc.tensor.matmul(
                            ps[:], w2p[:, c * H + hb * 128: c * H + (hb + 1) * 128],
                            h1T[:, c * BS:(c + 1) * BS],
                            start=(c == 0), stop=False)
                    nc.tensor.matmul(ps[:], b2r[:, hb * 128:(hb + 1) * 128],
                                     ones_row[:], start=False, stop=True)
                    nc.scalar.activation(h2T[:, hb * BS:(hb + 1) * BS], ps[:], ACTF.Relu)
                ps3 = mpps.tile([3, BS], f32, tag="mlp3")
                for c in range(4):
                    nc.tensor.matmul(ps3[:], w3p[:, c * D:(c + 1) * D],
                                     h2T[:, c * BS:(c + 1) * BS],
                                     start=(c == 0), stop=False)
                nc.tensor.matmul(ps3[:], b3r[:], ones_row[:], start=False, stop=True)
                nc.scalar.copy(xdT[:], ps3[:])

            # ---- pass A: bank-major cdist (bitwise-identical to pass B),
            #      PE-transposed to query-major, then max8 candidates --------
            with tc.tile_pool(name="pa", bufs=4) as pa, \
                 tc.tile_pool(name="paasm", bufs=2) as paasm, \
                 tc.tile_pool(name="paps", bufs=3, space="PSUM") as paps:
                asm = [None, None]
                for c in range(NCHUNK):
                    band, ci = divmod(c, CPB)
                    bp = 32 * band
                    gT_ps = paps.tile([128, BS], f32, tag="gt")
                    nc.tensor.matmul(
                        gT_ps[:],
                        bankp[bp:bp + 5, ci * 128:(ci + 1) * 128],
                        qrep[bp:bp + 5, :],
                        tile_position=(bp, 0))
                    gT_sb = pa.tile([128, BS], f32, tag="gtsb")
                    nc.scalar.copy(gT_sb[:], gT_ps[:])
                    nc.sync.dma_start(gdram_d[c * 128:(c + 1) * 128, :], gT_sb[:])
                    if c % 4 == 0:
                        asm = [paasm.tile([128, TILW], f32, name=f"asm{qb}",
                                          tag=f"asm{qb}") for qb in range(2)]
                    col = (c % 4) * 128
                    for qb in range(2):
                        tq_ps = paps.tile([128, 128], f32, tag="tq")
                        nc.tensor.transpose(
                            tq_ps[:], gT_sb[:, qb * 128:(qb + 1) * 128], ident[:])
                        nc.scalar.copy(asm[qb][:, col:col + 128], tq_ps[:])
                    if c % 4 == 3:
                        t = c // 4
                        for qb in range(2):
                            nc.vector.max(cand[qb][:, t * 8:(t + 1) * 8], asm[qb][:])
                # level-2: 13 rounds of top-8 extraction (destructive)
                for qb in range(2):
                    for r in range(ROUNDS):
                        if r < ROUNDS - 1:
                            r8 = pa.tile([128, 8], f32, tag="r8")
                            nc.vector.max(r8[:], cand[qb][:])
                            nc.vector.match_replace(cand[qb][:], r8[:], cand[qb][:], NEGBIG)
                        else:
                            nc.vector.max(r13[qb][:], cand[qb][:])
                    # tie count over the destroyed buffer (= count_le - 96)
                    nc.vector.tensor_scalar(
                        junk[:], cand[qb][:], r13[qb][:, 3:4], None,
                        OP.is_ge, OP.add, accum_out=cnt[qb][:])

            # ---- build negh2_rep / s_rep [128, BS] -----------------------
            with tc.tile_pool(name="rep", bufs=1) as rp, \
                 tc.tile_pool(name="repps", bufs=2, space="PSUM") as rpps:
                nh_row = rp.tile([1, BS], f32)
                for qb in range(2):
                    tp = rpps.tile([1, 128], f32, tag="tp")
                    nc.tensor.transpose(tp[:], r13[qb][:, 3:4], ident[:])
                    nc.scalar.copy(nh_row[:, qb * 128:(qb + 1) * 128], tp[:])
                ones_col = rp.tile([1, 128], f32)
                nc.vector.memset(ones_col[:], 1.0)
                bps = rpps.tile([128, BS], f32, tag="b")
                nc.tensor.matmul(bps[:], ones_col[:], nh_row[:])
                nc.scalar.copy(negh2_rep[:], bps[:])
                srm = rp.tile([128, BS], f32)
                nc.vector.tensor_scalar(srm[:], negh2_rep[:], -2.0, None, OP.mult)
                nc.vector.reciprocal(s_rep[:], srm[:])

            # ---- pass B: mask + weights + accumulation -------------------
            # gT streamed back from the DRAM scratch written in pass A
            # (bitwise-identical by construction; no PE recompute).
            with tc.tile_pool(name="pb", bufs=8) as pb, \
                 tc.tile_pool(name="pbacc", bufs=1, space="PSUM") as pbacc:
                u_acc = [pbacc.tile([128, 4], f32, name=f"uacc{b}", tag=f"uacc{b}")
                         for b in range(2)]
                for c in range(NCHUNK):
                    gT = pb.tile([128, BS], f32, tag="gt")
                    nc.sync.dma_start(gT[:], gdram_d[c * 128:(c + 1) * 128, :])
                    gts = pb.tile([128, BS], f32, tag="gts")
                    nc.vector.tensor_tensor(gts[:], gT[:], s_rep[:], OP.mult)
                    incl = pb.tile([128, BS], f32, tag="incl")
                    nc.vector.tensor_tensor(incl[:], gT[:], negh2_rep[:], OP.is_ge)
                    wt = pb.tile([128, BS], f32, tag="wt")
                    nc.scalar.activation(wt[:], gts[:], ACTF.Exp)
                    wtm = pb.tile([128, BS], f32, tag="wtm")
                    nc.gpsimd.tensor_tensor(wtm[:], wt[:], incl[:], OP.mult)
                    for qb in range(2):
                        nc.tensor.matmul(
                            u_acc[qb][:], wtm[:, qb * 128:(qb + 1) * 128],
                            vpack[:, c * 4:(c + 1) * 4],
                            start=(c == 0), stop=(c == NCHUNK - 1))
                for qb in range(2):
                    nc.scalar.copy(u4[qb][:], u_acc[qb][:])
                    tp3 = pbacc.tile([128, 3], f32, name=f"tp3{qb}", tag="tp3")
                    nc.tensor.transpose(tp3[:], xdT[:, qb * 128:(qb + 1) * 128],
                                        ident[:3, :3])
                    nc.scalar.copy(xdb[qb][:], tp3[:])

            # ---- metrics + output ---------------------------------------
            with tc.tile_pool(name="met", bufs=1) as mt:
                for qb in range(2):
                    den = mt.tile([128, 1], f32, tag="den")
                    nc.vector.tensor_scalar(den[:], u4[qb][:, 3:4], EPS_KNN, None, OP.add)
                    rec = mt.tile([128, 1], f32, tag="rec")
                    nc.vector.reciprocal(rec[:], den[:])
                    ut = mt.tile([128, 3], f32, tag="ut")
                    nc.vector.tensor_scalar(ut[:], u4[qb][:, 0:3], rec[:], None, OP.mult)
                    xd = xdb[qb]
                    prod = mt.tile([128, 3], f32, tag="prod")
                    nc.vector.tensor_tensor(prod[:], ut[:], xd[:], OP.mult)
                    dot = mt.tile([128, 1], f32, tag="dot")
                    nc.vector.tensor_reduce(dot[:], prod[:], mybir.AxisListType.X, OP.add)
                    uu = mt.tile([128, 3], f32, tag="uu")
                    nc.vector.tensor_tensor(uu[:], ut[:], ut[:], OP.mult)
                    nu2 = mt.tile([128, 1], f32, tag="nu2")
                    nc.vector.tensor_reduce(nu2[:], uu[:], mybir.AxisListType.X, OP.add)
                    nu = mt.tile([128, 1], f32, tag="nu")
                    nc.scalar.activation(nu[:], nu2[:], ACTF.Sqrt)
                    nc.vector.tensor_scalar(nu[:], nu[:], EPS_COS, None, OP.max)
                    xx = mt.tile([128, 3], f32, tag="xx")
                    nc.vector.tensor_tensor(xx[:], xd[:], xd[:], OP.mult)
                    nd2 = mt.tile([128, 1], f32, tag="nd2")
                    nc.vector.tensor_reduce(nd2[:], xx[:], mybir.AxisListType.X, OP.add)
                    nd = mt.tile([128, 1], f32, tag="nd")
                    nc.scalar.activation(nd[:], nd2[:], ACTF.Sqrt)
                    nc.vector.tensor_scalar(nd[:], nd[:], EPS_COS, None, OP.max)
                    nprod = mt.tile([128, 1], f32, tag="npr")
                    nc.vector.tensor_tensor(nprod[:], nu[:], nd[:], OP.mult)
                    nrec = mt.tile([128, 1], f32, tag="nrec")
                    nc.vector.reciprocal(nrec[:], nprod[:])
                    cosv = mt.tile([128, 1], f32, tag="cosv")
                    nc.vector.tensor_tensor(cosv[:], dot[:], nrec[:], OP.mult)
                    cosd = mt.tile([128, 1], f32, tag="cosd")
                    nc.vector.tensor_scalar(cosd[:], cosv[:], -1.0, 1.0, OP.mult, OP.add)
                    diff = mt.tile([128, 3], f32, tag="diff")
                    nc.vector.tensor_tensor(diff[:], ut[:], xd[:], OP.subtract)
                    dsq = mt.tile([128, 3], f32, tag="dsq")
                    nc.vector.tensor_tensor(dsq[:], diff[:], diff[:], OP.mult)
                    l2 = mt.tile([128, 1], f32, tag="l2")
                    nc.vector.tensor_reduce(l2[:], dsq[:], mybir.AxisListType.X, OP.add)

                    ot = mt.tile([128, 6], f32, tag="ot")
                    nc.vector.tensor_copy(ot[:, 0:3], xd[:])
                    nc.vector.tensor_copy(ot[:, 3:4], cosd[:])
                    nc.vector.tensor_copy(ot[:, 4:5], cosd[:])
                    nc.vector.tensor_copy(ot[:, 5:6], l2[:])
                    nc.sync.dma_start(out_d[qb * 128:(qb + 1) * 128, :], ot[:])

                    at = mt.tile([128, 9], f32, tag="at")
                    nc.vector.tensor_copy(at[:, 0:1], cnt[qb][:])
                    nc.vector.tensor_copy(at[:, 1:9], r13[qb][:])
                    nc.sync.dma_start(aux_d[qb * 128:(qb + 1) * 128, :], at[:])

    nc.finalize()
    return nc


def _host_prep(inputs):
    """Build all device input arrays (shared + per-core)."""
    z = np.asarray(inputs["z"], np.float32)
    t = np.float32(np.asarray(inputs["t"]))
    x0 = np.asarray(inputs["x0"], np.float32)
    x1 = np.asarray(inputs["x1"], np.float32)
    v0 = np.asarray(inputs["v0"], np.float32)
    v1 = np.asarray(inputs["v1"], np.float32)
    W1 = np.asarray(inputs["W1"], np.float32)
    b1 = np.asarray(inputs["b1"], np.float32)
    W2 = np.asarray(inputs["W2"], np.float32)
    b2 = np.asarray(inputs["b2"], np.float32)
    W3 = np.asarray(inputs["W3"], np.float32)
    b3 = np.asarray(inputs["b3"], np.float32)

    xb = np.concatenate([x0, x1], 0)
    vb = np.concatenate([v0, v1], 0)
    xbp = np.full((MP, D), PADC, np.float32)
    xbp[:M] = xb
    nb = (xbp * xbp).sum(1).astype(np.float32)

    bankp = np.zeros((128, CBAND), np.float32)
    for band in range(4):
        cols = slice(band * CBAND, (band + 1) * CBAND)
        bankp[32 * band + 0] = xbp[cols, 0]
        bankp[32 * band + 1] = xbp[cols, 1]
        bankp[32 * band + 2] = xbp[cols, 2]
        bankp[32 * band + 3] = nb[cols]
        bankp[32 * band + 4] = 1.0

    vpk = np.zeros((MP, 4), np.float32)
    vpk[:M, :3] = vb
    vpk[:M, 3] = 1.0
    vpack = np.ascontiguousarray(
        vpk.reshape(NCHUNK, 128, 4).transpose(1, 0, 2).reshape(128, NCHUNK * 4))

    w1aug = np.concatenate([W1, b1[None, :]], 0).astype(np.float32)       # [5, 512]
    w2p = np.ascontiguousarray(
        W2.reshape(4, 128, H).transpose(1, 0, 2).reshape(128, 4 * H))
    w3p = np.ascontiguousarray(
        W3.reshape(4, 128, D).transpose(1, 0, 2).reshape(128, 4 * D))
    ident = np.eye(128, dtype=np.float32)

    shared = dict(bankp=bankp, vpack=vpack, w1aug=w1aug, w2p=w2p,
                  b2row=b2[None, :].astype(np.float32), w3p=w3p,
                  b3row=b3[None, :].astype(np.float32), ident=ident)

    in_maps = []
    for c in range(NCORES):
        xq = z[c * BS:(c + 1) * BS, :D]
        nq = (xq * xq).sum(1).astype(np.float32)
        qrow = np.stack([2 * xq[:, 0], 2 * xq[:, 1], 2 * xq[:, 2],
                         -np.ones(BS, np.float32), -nq], 0).astype(np.float32)
        qrep = np.zeros((128, BS), np.float32)
        for band in range(4):
            qrep[32 * band:32 * band + 5] = qrow
        xint = np.stack([xq[:, 0], xq[:, 1], xq[:, 2],
                         np.full(BS, t, np.float32),
                         np.ones(BS, np.float32)], 0).astype(np.float32)
        in_maps.append(dict(shared, qrep=qrep, xint=xint))
    return in_maps


def _host_fix(out, aux, inputs):
    """Recompute rows with exact/near ties at the rank-100 boundary using
    jax on CPU, replicating the reference bit-for-bit (incl. top_k order)."""
    count_le = aux[:, 0] + 96.0
    g100 = aux[:, 1 + 3]
    g101 = aux[:, 1 + 4]
    flags = (count_le != float(KNN)) | ((g100 - g101) < GAP_THRESH)
    idx = np.nonzero(flags)[0]
    if len(idx) == 0:
        return out
    import jax
    import jax.numpy as jnp
    cpu = jax.devices("cpu")[0]
    with jax.default_device(cpu):
        z = jnp.asarray(np.asarray(inputs["z"], np.float32)[idx])
        t = jnp.float32(np.asarray(inputs["t"]))
        x0 = jnp.asarray(np.asarray(inputs["x0"], np.float32))
        x1 = jnp.asarray(np.asarray(inputs["x1"], np.float32))
        v0 = jnp.asarray(np.asarray(inputs["v0"], np.float32))
        v1 = jnp.asarray(np.asarray(inputs["v1"], np.float32))
        W1 = jnp.asarray(np.asarray(inputs["W1"], np.float32))
        b1 = jnp.asarray(np.asarray(inputs["b1"], np.float32))
        W2 = jnp.asarray(np.asarray(inputs["W2"], np.float32))
        b2 = jnp.asarray(np.asarray(inputs["b2"], np.float32))
        W3 = jnp.asarray(np.asarray(inputs["W3"], np.float32))
        b3 = jnp.asarray(np.asarray(inputs["b3"], np.float32))

        x = z[:, :-3]
        nB = x.shape[0]
        t_col = jnp.full((nB, 1), t, dtype=x.dtype)
        h = jax.nn.relu(jnp.concatenate([x, t_col], axis=1) @ W1 + b1)
        h = jax.nn.relu(h @ W2 + b2)
        x_dot = h @ W3 + b3
        xcat = jnp.concatenate([x0, x1], axis=0)
        vcat = jnp.concatenate([v0, v1], axis=0)
        d2 = ((x * x).sum(1, keepdims=True) + (xcat * xcat).sum(1)[None, :]
              - 2.0 * x @ xcat.T)
        dists = jnp.sqrt(jnp.maximum(d2, 0.0))
        neg_d, knn_idx = jax.lax.top_k(-dists, KNN)
        knn_dists = -neg_d
        hh = jnp.maximum(knn_dists[:, -1:], EPS_KNN)
        w = jnp.exp(-knn_dists ** 2 / (2.0 * hh ** 2))
        w = w / (w.sum(1, keepdims=True) + EPS_KNN)
        v_knn = vcat[knn_idx]
        u_t = jnp.einsum("bk,bkd->bd", w, v_knn)
        nu = jnp.maximum(jnp.linalg.norm(u_t, axis=1), EPS_COS)
        nd = jnp.maximum(jnp.linalg.norm(x_dot, axis=1), EPS_COS)
        cos_dist = 1.0 - (u_t * x_dot).sum(1) / (nu * nd)
        l2_sq = ((u_t - x_dot) ** 2).sum(1)
        fix = jnp.concatenate(
            [x_dot, cos_dist[:, None], cos_dist[:, None], l2_sq[:, None]], axis=1)
        out[idx] = np.asarray(fix)
    return out


def _setup_trace():
    """Register the NTFF profile hook (missing from this image's antenv stub)
    so run_bass_kernel_spmd(trace=True) can measure device exec time."""
    try:
        import sys
        import types
        if "antenv.axon_hooks" not in sys.modules:
            import antenv
            mod = types.ModuleType("antenv.axon_hooks")
            mod._hook = None
            mod.set_axon_ntff_profile_hook = lambda h: setattr(mod, "_hook", h)
            mod.get_axon_ntff_profile_hook = lambda: mod._hook
            sys.modules["antenv.axon_hooks"] = mod
            antenv.axon_hooks = mod
        import antenv.axon_hooks as ah
        if ah.get_axon_ntff_profile_hook() is None:
            from trn_agent_boot.trn_boot import _ntff_profile_via_ctypes
            ah.set_axon_ntff_profile_hook(
                _ntff_profile_via_ctypes("/opt/axon/libaxon_pjrt.so"))
        from concourse import bass_utils as bu
        bu.upload_artifacts = lambda tmpdir: tmpdir   # no fish bucket here
        return True
    except Exception as e:                            # pragma: no cover
        print("trace setup failed:", e)
        return False


def kernel(**inputs):
    from concourse.bass_utils import run_bass_kernel_spmd

    assert int(np.asarray(inputs["k"])) == KNN
    if "nc" not in _prog_cache:
        _prog_cache["nc"] = _build_program()
    nc = _prog_cache["nc"]

    in_maps = _host_prep(inputs)
    trace = os.environ.get("KNN_TRACE") == "1" and _setup_trace()
    try:
        res = run_bass_kernel_spmd(nc, in_maps, list(range(NCORES)), trace=trace)
    except Exception:
        if not trace:
            raise
        res = run_bass_kernel_spmd(nc, in_maps, list(range(NCORES)), trace=False)
    if trace:
        _prog_cache["last_result"] = res

    out = np.concatenate([res.results[c]["out"] for c in range(NCORES)], 0)
    aux = np.concatenate([res.results[c]["aux"] for c in range(NCORES)], 0)
    _prog_cache["last_aux"] = aux
    out = _host_fix(out, aux, inputs)
    return out.astype(np.float32)



# revision 3
# speedup vs baseline: 1.4589x; 1.4589x over previous
"""Trainium2 Bass kernel for nn_CurlyWrapperWithMetricsCFD (retrieval_knn).

Data-parallel over the query batch B=2048 across 8 NeuronCores (256 queries
per core). The x/v banks (2x50000x3) and MLP weights are replicated.

v3 design (no DRAM scratch, no PE transposes in the hot path):
  MLP      : 3-layer MLP on transposed activations (PE + ACT relu).
  Pass A   : query-major cdist g[q, j] = -d2 via K=5 augmented matmuls with
             the QUERIES stationary (a handful of LDWEIGHTS total); per
             1024-bank tile top-8 (DVE max8 straight from PSUM) -> candidate
             buffer [128, 800]; 13 rounds max8+match_replace -> exact
             top-104; g100 = 100th best (h2 = -g100).
  Mid      : s = -1/(2*g100) per query; qs = qrep * s (scaled queries) so the
             pass-B matmul emits s*g directly and the mask threshold becomes
             the CONSTANT -0.5 (works in bank-major without per-query rows).
  Pass B   : bank-stationary recompute of s*g per 128-bank chunk; w=exp(s*g)
             (ACT, unmasked - far points underflow to 0); mask=[s*g>=th_m]
             (DVE, exact fp32 boundary); accumulation uses the split
             u = sum relu(w-th_w)*vaug + th_w * sum mask*vaug
             as two bf16 matmuls sharing a stationary vaug chunk; the mask
             sum's 4th component is the exact inclusion count.
  Metrics  : u_t = u[:3]/(u[3]+1e-12); cos_dist; l2 -> out [256, 6].
Host       : rows whose boundary is ambiguous (count != 100 or scaled gap
             s*(g100-g101) < FLAG_SCALED) are recomputed exactly with
             jax-CPU replicating the reference. Typically ~60-150/2048.

Self-contained: hardcodes all shapes for B=2048, N=50000, D=3, H=512, k=100.
"""

import os
import numpy as np

# ---------------------------------------------------------------------------
# problem constants (hardcoded per spec)
B = 2048
N = 50000
D = 3
H = 512
KNN = 100
NCORES = 8
BS = B // NCORES            # 256 queries per core
M = 2 * N                   # 100000 bank points
MP = 102400                 # padded bank (4 bands x 25600)
CBAND = MP // 4             # 25600 columns per partition band
TILA = 1024                 # pass-A tile width (2 psum banks)
NTA = MP // TILA            # 100 tiles
TPB = CBAND // TILA         # 25 pass-A tiles per band
NCAND = NTA * 8             # 800 candidates per query
ROUNDS = 13                 # 13*8 = 104 >= 100
NCHUNK = MP // 128          # 800 pass-B chunks
CPB = CBAND // 128          # 200 pass-B chunks per band
NEGBIG = -3.0e38            # match_replace fill
PADC = 1000.0               # pad coordinate -> g ~ -3e6, never selected
EPS_KNN = 1e-12
EPS_COS = 1e-8
MARGIN_M = 3.0e-5           # mask threshold margin (scaled domain)
THETA_M = -0.5 - MARGIN_M   # mask: s*g >= THETA_M
THETA_W = float(np.exp(-0.5 - MARGIN_M) * (1.0 - 2.0 ** -8))  # relu split pt
FLAG_SCALED = 1.5e-4        # host-fix rows with s*(g100-g101) below this

_prog_cache = {}


def _build_program():
    import concourse.bass as bass
    import concourse.bacc as bacc
    import concourse.mybir as mybir
    from concourse import tile

    f32 = mybir.dt.float32
    bf16 = mybir.dt.bfloat16
    OP = mybir.AluOpType
    ACTF = mybir.ActivationFunctionType

    nc = bacc.Bacc("TRN2", target_bir_lowering=False, debug=False,
                   num_devices=NCORES)

    # ---- dram parameters -------------------------------------------------
    bankp_d = nc.declare_dram_parameter("bankp", [128, CBAND], f32, isOutput=False)
    vaug_d = nc.declare_dram_parameter("vaug", [128, NCHUNK * 4], bf16, isOutput=False)
    qrep_d = nc.declare_dram_parameter("qrep", [128, BS], f32, isOutput=False)
    xint_d = nc.declare_dram_parameter("xint", [5, BS], f32, isOutput=False)
    w1a_d = nc.declare_dram_parameter("w1aug", [5, H], f32, isOutput=False)
    w2p_d = nc.declare_dram_parameter("w2p", [128, 4 * H], f32, isOutput=False)
    b2r_d = nc.declare_dram_parameter("b2row", [1, H], f32, isOutput=False)
    w3p_d = nc.declare_dram_parameter("w3p", [128, 4 * D], f32, isOutput=False)
    b3r_d = nc.declare_dram_parameter("b3row", [1, D], f32, isOutput=False)
    iden_d = nc.declare_dram_parameter("ident", [128, 128], f32, isOutput=False)
    out_d = nc.declare_dram_parameter("out", [BS, 6], f32, isOutput=True)
    aux_d = nc.declare_dram_parameter("aux", [BS, 9], f32, isOutput=True)

    with tile.TileContext(nc) as tc:
        from contextlib import ExitStack
        with ExitStack() as ctx:
            cp = ctx.enter_context(tc.tile_pool(name="const", bufs=1))
            # ---- constant loads ------------------------------------------
            bankp = cp.tile([128, CBAND], f32)
            nc.sync.dma_start(bankp[:], bankp_d[:])
            vaug = cp.tile([128, NCHUNK * 4], bf16)
            nc.sync.dma_start(vaug[:], vaug_d[:])
            qrep = cp.tile([128, BS], f32)
            nc.sync.dma_start(qrep[:], qrep_d[:])
            ident = cp.tile([128, 128], f32)
            nc.sync.dma_start(ident[:], iden_d[:])
            ones_row = cp.tile([1, BS], f32)
            nc.vector.memset(ones_row[:], 1.0)

            # persistent small tiles
            xdT = cp.tile([3, BS], f32)            # MLP output, transposed
            cand = [cp.tile([128, NCAND], f32, name=f"cand{b}", tag=f"cand{b}")
                    for b in range(2)]
            r13 = [cp.tile([128, 8], f32, name=f"r13{b}", tag=f"r13{b}") for b in range(2)]
            qs = cp.tile([128, BS], f32)           # scaled queries
            uaT = [cp.tile([128, 4], f32, name=f"uaT{b}", tag=f"uaT{b}") for b in range(2)]
            ubT = [cp.tile([128, 4], f32, name=f"ubT{b}", tag=f"ubT{b}") for b in range(2)]
            xdb = [cp.tile([128, 3], f32, name=f"xdb{b}", tag=f"xdb{b}") for b in range(2)]

            # ---- MLP (transposed activations) ----------------------------
            with tc.tile_pool(name="mlp", bufs=1) as mp, \
                 tc.tile_pool(name="mlpps", bufs=2, space="PSUM") as mpps:
                xint = mp.tile([5, BS], f32)
                nc.sync.dma_start(xint[:], xint_d[:])
                w1a = mp.tile([5, H], f32)
                nc.sync.dma_start(w1a[:], w1a_d[:])
                w2p = mp.tile([128, 4 * H], f32)
                nc.sync.dma_start(w2p[:], w2p_d[:])
                b2r = mp.tile([1, H], f32)
                nc.sync.dma_start(b2r[:], b2r_d[:])
                w3p = mp.tile([128, 4 * D], f32)
                nc.sync.dma_start(w3p[:], w3p_d[:])
                b3r = mp.tile([1, D], f32)
                nc.sync.dma_start(b3r[:], b3r_d[:])

                h1T = mp.tile([128, 4 * BS], f32)   # [hb*BS + q]
                for hb in range(4):
                    ps = mpps.tile([128, BS], f32, tag="mlp1")
                    nc.tensor.matmul(ps[:], w1a[:, hb * 128:(hb + 1) * 128], xint[:])
                    nc.scalar.activation(h1T[:, hb * BS:(hb + 1) * BS], ps[:], ACTF.Relu)
                h2T = mp.tile([128, 4 * BS], f32)
                for hb in range(4):
                    ps = mpps.tile([128, BS], f32, tag="mlp2")
                    for c in range(4):
                        nc.tensor.matmul(
                            ps[:], w2p[:, c * H + hb * 128: c * H + (hb + 1) * 128],
                            h1T[:, c * BS:(c + 1) * BS],
                            start=(c == 0), stop=False)
                    nc.tensor.matmul(ps[:], b2r[:, hb * 128:(hb + 1) * 128],
                                     ones_row[:], start=False, stop=True)
                    nc.scalar.activation(h2T[:, hb * BS:(hb + 1) * BS], ps[:], ACTF.Relu)
                ps3 = mpps.tile([3, BS], f32, tag="mlp3")
                for c in range(4):
                    nc.tensor.matmul(ps3[:], w3p[:, c * D:(c + 1) * D],
                                     h2T[:, c * BS:(c + 1) * BS],
                                     start=(c == 0), stop=False)
                nc.tensor.matmul(ps3[:], b3r[:], ones_row[:], start=False, stop=True)
                nc.scalar.copy(xdT[:], ps3[:])

            # ---- pass A: query-major cdist + per-1024 top-8 + rounds -----
            with tc.tile_pool(name="pa", bufs=2) as pa, \
                 tc.tile_pool(name="paps", bufs=3, space="PSUM") as paps:
                for qb in range(2):
                    qsl = slice(qb * 128, (qb + 1) * 128)
                    for t in range(NTA):
                        band, ci = divmod(t, TPB)
                        bp = 32 * band
                        p = paps.tile([128, TILA], f32, tag="ga")
                        nc.tensor.matmul(
                            p[:, 0:512],
                            qrep[bp:bp + 5, qsl],
                            bankp[bp:bp + 5, ci * TILA: ci * TILA + 512],
                            tile_position=(bp, 0))
                        nc.tensor.matmul(
                            p[:, 512:1024],
                            qrep[bp:bp + 5, qsl],
                            bankp[bp:bp + 5, ci * TILA + 512: (ci + 1) * TILA],
                            tile_position=(bp, 0))
                        nc.vector.max(cand[qb][:, t * 8:(t + 1) * 8], p[:])
                    for r in range(ROUNDS):
                        if r < ROUNDS - 1:
                            r8 = pa.tile([128, 8], f32, tag="r8")
                            nc.vector.max(r8[:], cand[qb][:])
                            nc.vector.match_replace(cand[qb][:], r8[:], cand[qb][:], NEGBIG)
                        else:
                            nc.vector.max(r13[qb][:], cand[qb][:])

            # ---- mid: qs = qrep * s,  s = -1/(2*g100) per query ----------
            with tc.tile_pool(name="rep", bufs=1) as rp, \
                 tc.tile_pool(name="repps", bufs=2, space="PSUM") as rpps:
                s_row = rp.tile([1, BS], f32)
                for qb in range(2):
                    srm = rp.tile([128, 1], f32, tag="srm")
                    nc.vector.tensor_scalar(srm[:], r13[qb][:, 3:4], -2.0, None, OP.mult)
                    s_col = rp.tile([128, 1], f32, tag="scol")
                    nc.vector.reciprocal(s_col[:], srm[:])
                    tp = rpps.tile([1, 128], f32, tag="tp")
                    nc.tensor.transpose(tp[:], s_col[:], ident[:])
                    nc.scalar.copy(s_row[:, qb * 128:(qb + 1) * 128], tp[:])
                ones_col = rp.tile([1, 128], f32)
                nc.vector.memset(ones_col[:], 1.0)
                bps = rpps.tile([128, BS], f32, tag="b")
                nc.tensor.matmul(bps[:], ones_col[:], s_row[:])
                s_rep = rp.tile([128, BS], f32)
                nc.scalar.copy(s_rep[:], bps[:])
                nc.vector.tensor_tensor(qs[:], qrep[:], s_rep[:], OP.mult)

            # ---- pass B: bank-stationary recompute + masked accumulation -
            with tc.tile_pool(name="pb", bufs=6) as pb, \
                 tc.tile_pool(name="pbgs", bufs=3, space="PSUM") as pbgs, \
                 tc.tile_pool(name="pbacc", bufs=1, space="PSUM") as pbacc:
                u_a = pbacc.tile([4, 512], f32, name="u_a", tag="u_a")
                u_b = pbacc.tile([4, 512], f32, name="u_b", tag="u_b")
                for c in range(NCHUNK):
                    band, ci = divmod(c, CPB)
                    bp = 32 * band
                    gs = pbgs.tile([128, 512], f32, tag="gs")
                    nc.tensor.matmul(
                        gs[:, 0:BS],
                        bankp[bp:bp + 5, ci * 128:(ci + 1) * 128],
                        qs[bp:bp + 5, :],
                        tile_position=(bp, 0))
                    wu = pb.tile([128, BS], bf16, tag="wu")
                    nc.scalar.activation(wu[:], gs[:, 0:BS], ACTF.Exp)
                    msk = pb.tile([128, BS], bf16, tag="msk")
                    nc.vector.tensor_scalar(msk[:], gs[:, 0:BS], THETA_M, None, OP.is_ge)
                    wr = pb.tile([128, BS], bf16, tag="wr")
                    nc.vector.tensor_scalar(wr[:], wu[:], THETA_W, 0.0,
                                            OP.subtract, OP.max)
                    nc.tensor.matmul(u_a[:, 0:BS], vaug[:, 4 * c:4 * c + 4], wr[:],
                                     start=(c == 0), stop=(c == NCHUNK - 1))
                    nc.tensor.matmul(u_b[:, 0:BS], vaug[:, 4 * c:4 * c + 4], msk[:],
                                     start=(c == 0), stop=(c == NCHUNK - 1))
                # combine: u = u_a + THETA_W * u_b   (still [4, 256])
                ua_sb = cp.tile([4, BS], f32)
                nc.scalar.copy(ua_sb[:], u_a[:, 0:BS])
                ub_sb = cp.tile([4, BS], f32)
                nc.scalar.copy(ub_sb[:], u_b[:, 0:BS])
                uc_sb = cp.tile([4, BS], f32)
                nc.vector.tensor_scalar(uc_sb[:], ub_sb[:], THETA_W, None, OP.mult)
                nc.vector.tensor_tensor(uc_sb[:], uc_sb[:], ua_sb[:], OP.add)

            with tc.tile_pool(name="fint", bufs=2, space="PSUM") as ftp:
                for qb in range(2):
                    qsl = slice(qb * 128, (qb + 1) * 128)
                    tpu = ftp.tile([128, 4], f32, tag="tp")
                    nc.tensor.transpose(tpu[:], uc_sb[:, qsl], ident[:4, :4])
                    nc.scalar.copy(uaT[qb][:], tpu[:])
                    tpb_ = ftp.tile([128, 4], f32, tag="tp")
                    nc.tensor.transpose(tpb_[:], ub_sb[:, qsl], ident[:4, :4])
                    nc.scalar.copy(ubT[qb][:], tpb_[:])
                    tp3 = ftp.tile([128, 3], f32, tag="tp")
                    nc.tensor.transpose(tp3[:], xdT[:, qsl], ident[:3, :3])
                    nc.scalar.copy(xdb[qb][:], tp3[:])

            # ---- metrics + output ---------------------------------------
            with tc.tile_pool(name="met", bufs=1) as mt:
                for qb in range(2):
                    u4 = uaT[qb]
                    den = mt.tile([128, 1], f32, tag="den")
                    nc.vector.tensor_scalar(den[:], u4[:, 3:4], EPS_KNN, None, OP.add)
                    rec = mt.tile([128, 1], f32, tag="rec")
                    nc.vector.reciprocal(rec[:], den[:])
                    ut = mt.tile([128, 3], f32, tag="ut")
                    nc.vector.tensor_scalar(ut[:], u4[:, 0:3], rec[:], None, OP.mult)
                    xd = xdb[qb]
                    prod = mt.tile([128, 3], f32, tag="prod")
                    nc.vector.tensor_tensor(prod[:], ut[:], xd[:], OP.mult)
                    dot = mt.tile([128, 1], f32, tag="dot")
                    nc.vector.tensor_reduce(dot[:], prod[:], mybir.AxisListType.X, OP.add)
                    uu = mt.tile([128, 3], f32, tag="uu")
                    nc.vector.tensor_tensor(uu[:], ut[:], ut[:], OP.mult)
                    nu2 = mt.tile([128, 1], f32, tag="nu2")
                    nc.vector.tensor_reduce(nu2[:], uu[:], mybir.AxisListType.X, OP.add)
                    nu = mt.tile([128, 1], f32, tag="nu")
                    nc.scalar.activation(nu[:], nu2[:], ACTF.Sqrt)
                    nc.vector.tensor_scalar(nu[:], nu[:], EPS_COS, None, OP.max)
                    xx = mt.tile([128, 3], f32, tag="xx")
                    nc.vector.tensor_tensor(xx[:], xd[:], xd[:], OP.mult)
                    nd2 = mt.tile([128, 1], f32, tag="nd2")
                    nc.vector.tensor_reduce(nd2[:], xx[:], mybir.AxisListType.X, OP.add)
                    nd = mt.tile([128, 1], f32, tag="nd")
                    nc.scalar.activation(nd[:], nd2[:], ACTF.Sqrt)
                    nc.vector.tensor_scalar(nd[:], nd[:], EPS_COS, None, OP.max)
                    nprod = mt.tile([128, 1], f32, tag="npr")
                    nc.vector.tensor_tensor(nprod[:], nu[:], nd[:], OP.mult)
                    nrec = mt.tile([128, 1], f32, tag="nrec")
                    nc.vector.reciprocal(nrec[:], nprod[:])
                    cosv = mt.tile([128, 1], f32, tag="cosv")
                    nc.vector.tensor_tensor(cosv[:], dot[:], nrec[:], OP.mult)
                    cosd = mt.tile([128, 1], f32, tag="cosd")
                    nc.vector.tensor_scalar(cosd[:], cosv[:], -1.0, 1.0, OP.mult, OP.add)
                    diff = mt.tile([128, 3], f32, tag="diff")
                    nc.vector.tensor_tensor(diff[:], ut[:], xd[:], OP.subtract)
                    dsq = mt.tile([128, 3], f32, tag="dsq")
                    nc.vector.tensor_tensor(dsq[:], diff[:], diff[:], OP.mult)
                    l2 = mt.tile([128, 1], f32, tag="l2")
                    nc.vector.tensor_reduce(l2[:], dsq[:], mybir.AxisListType.X, OP.add)

                    ot = mt.tile([128, 6], f32, tag="ot")
                    nc.vector.tensor_copy(ot[:, 0:3], xd[:])
                    nc.vector.tensor_copy(ot[:, 3:4], cosd[:])
                    nc.vector.tensor_copy(ot[:, 4:5], cosd[:])
                    nc.vector.tensor_copy(ot[:, 5:6], l2[:])
                    nc.sync.dma_start(out_d[qb * 128:(qb + 1) * 128, :], ot[:])

                    at = mt.tile([128, 9], f32, tag="at")
                    nc.vector.tensor_copy(at[:, 0:1], ubT[qb][:, 3:4])
                    nc.vector.tensor_copy(at[:, 1:9], r13[qb][:])
                    nc.sync.dma_start(aux_d[qb * 128:(qb + 1) * 128, :], at[:])

    nc.finalize()
    return nc


def _host_prep(inputs):
    """Build all device input arrays (shared + per-core)."""
    z = np.asarray(inputs["z"], np.float32)
    t = np.float32(np.asarray(inputs["t"]))
    x0 = np.asarray(inputs["x0"], np.float32)
    x1 = np.asarray(inputs["x1"], np.float32)
    v0 = np.asarray(inputs["v0"], np.float32)
    v1 = np.asarray(inputs["v1"], np.float32)
    W1 = np.asarray(inputs["W1"], np.float32)
    b1 = np.asarray(inputs["b1"], np.float32)
    W2 = np.asarray(inputs["W2"], np.float32)
    b2 = np.asarray(inputs["b2"], np.float32)
    W3 = np.asarray(inputs["W3"], np.float32)
    b3 = np.asarray(inputs["b3"], np.float32)
    import ml_dtypes
    bf16 = ml_dtypes.bfloat16

    xb = np.concatenate([x0, x1], 0)
    vb = np.concatenate([v0, v1], 0)
    xbp = np.full((MP, D), PADC, np.float32)
    xbp[:M] = xb
    nb = (xbp * xbp).sum(1).astype(np.float32)

    bankp = np.zeros((128, CBAND), np.float32)
    for band in range(4):
        cols = slice(band * CBAND, (band + 1) * CBAND)
        bankp[32 * band + 0] = xbp[cols, 0]
        bankp[32 * band + 1] = xbp[cols, 1]
        bankp[32 * band + 2] = xbp[cols, 2]
        bankp[32 * band + 3] = nb[cols]
        bankp[32 * band + 4] = 1.0

    vpk = np.zeros((MP, 4), np.float32)
    vpk[:M, :3] = vb
    vpk[:M, 3] = 1.0
    vaug = np.ascontiguousarray(
        vpk.reshape(NCHUNK, 128, 4).transpose(1, 0, 2).reshape(128, NCHUNK * 4)
    ).astype(bf16)

    w1aug = np.concatenate([W1, b1[None, :]], 0).astype(np.float32)       # [5, 512]
    w2p = np.ascontiguousarray(
        W2.reshape(4, 128, H).transpose(1, 0, 2).reshape(128, 4 * H))
    w3p = np.ascontiguousarray(
        W3.reshape(4, 128, D).transpose(1, 0, 2).reshape(128, 4 * D))
    ident = np.eye(128, dtype=np.float32)

    shared = dict(bankp=bankp, vaug=vaug, w1aug=w1aug, w2p=w2p,
                  b2row=b2[None, :].astype(np.float32), w3p=w3p,
                  b3row=b3[None, :].astype(np.float32), ident=ident)

    in_maps = []
    for c in range(NCORES):
        xq = z[c * BS:(c + 1) * BS, :D]
        nq = (xq * xq).sum(1).astype(np.float32)
        qrow = np.stack([2 * xq[:, 0], 2 * xq[:, 1], 2 * xq[:, 2],
                         -np.ones(BS, np.float32), -nq], 0).astype(np.float32)
        qrep = np.zeros((128, BS), np.float32)
        for band in range(4):
            qrep[32 * band:32 * band + 5] = qrow
        xint = np.stack([xq[:, 0], xq[:, 1], xq[:, 2],
                         np.full(BS, t, np.float32),
                         np.ones(BS, np.float32)], 0).astype(np.float32)
        in_maps.append(dict(shared, qrep=qrep, xint=xint))
    return in_maps


def _host_fix(out, aux, inputs):
    """Recompute rows with ambiguous rank-100 boundaries using jax on CPU,
    replicating the reference bit-for-bit (incl. top_k order)."""
    count = aux[:, 0]
    g100 = aux[:, 1 + 3]
    g101 = aux[:, 1 + 4]
    with np.errstate(divide="ignore", invalid="ignore"):
        s = -1.0 / (2.0 * g100)
        sgap = s * (g100 - g101)
    flags = (count != float(KNN)) | (sgap < FLAG_SCALED) | ~np.isfinite(sgap) \
        | (g100 >= -1e-9)
    idx = np.nonzero(flags)[0]
    if len(idx) == 0:
        return out
    import jax
    import jax.numpy as jnp
    cpu = jax.devices("cpu")[0]
    with jax.default_device(cpu):
        z = jnp.asarray(np.asarray(inputs["z"], np.float32)[idx])
        t = jnp.float32(np.asarray(inputs["t"]))
        x0 = jnp.asarray(np.asarray(inputs["x0"], np.float32))
        x1 = jnp.asarray(np.asarray(inputs["x1"], np.float32))
        v0 = jnp.asarray(np.asarray(inputs["v0"], np.float32))
        v1 = jnp.asarray(np.asarray(inputs["v1"], np.float32))
        W1 = jnp.asarray(np.asarray(inputs["W1"], np.float32))
        b1 = jnp.asarray(np.asarray(inputs["b1"], np.float32))
        W2 = jnp.asarray(np.asarray(inputs["W2"], np.float32))
        b2 = jnp.asarray(np.asarray(inputs["b2"], np.float32))
        W3 = jnp.asarray(np.asarray(inputs["W3"], np.float32))
        b3 = jnp.asarray(np.asarray(inputs["b3"], np.float32))

        x = z[:, :-3]
        nB = x.shape[0]
        t_col = jnp.full((nB, 1), t, dtype=x.dtype)
        h = jax.nn.relu(jnp.concatenate([x, t_col], axis=1) @ W1 + b1)
        h = jax.nn.relu(h @ W2 + b2)
        x_dot = h @ W3 + b3
        xcat = jnp.concatenate([x0, x1], axis=0)
        vcat = jnp.concatenate([v0, v1], axis=0)
        d2 = ((x * x).sum(1, keepdims=True) + (xcat * xcat).sum(1)[None, :]
              - 2.0 * x @ xcat.T)
        dists = jnp.sqrt(jnp.maximum(d2, 0.0))
        neg_d, knn_idx = jax.lax.top_k(-dists, KNN)
        knn_dists = -neg_d
        hh = jnp.maximum(knn_dists[:, -1:], EPS_KNN)
        w = jnp.exp(-knn_dists ** 2 / (2.0 * hh ** 2))
        w = w / (w.sum(1, keepdims=True) + EPS_KNN)
        v_knn = vcat[knn_idx]
        u_t = jnp.einsum("bk,bkd->bd", w, v_knn)
        nu = jnp.maximum(jnp.linalg.norm(u_t, axis=1), EPS_COS)
        nd = jnp.maximum(jnp.linalg.norm(x_dot, axis=1), EPS_COS)
        cos_dist = 1.0 - (u_t * x_dot).sum(1) / (nu * nd)
        l2_sq = ((u_t - x_dot) ** 2).sum(1)
        fix = jnp.concatenate(
            [x_dot, cos_dist[:, None], cos_dist[:, None], l2_sq[:, None]], axis=1)
        out[idx] = np.asarray(fix)
    return out


def _setup_trace():
    """Register the NTFF profile hook (missing from this image's antenv stub)
    so run_bass_kernel_spmd(trace=True) can measure device exec time."""
    try:
        import sys
        import types
        if "antenv.axon_hooks" not in sys.modules:
            import antenv
            mod = types.ModuleType("antenv.axon_hooks")
            mod._hook = None
            mod.set_axon_ntff_profile_hook = lambda h: setattr(mod, "_hook", h)
            mod.get_axon_ntff_profile_hook = lambda: mod._hook
            sys.modules["antenv.axon_hooks"] = mod
            antenv.axon_hooks = mod
        import antenv.axon_hooks as ah
        if ah.get_axon_ntff_profile_hook() is None:
            from trn_agent_boot.trn_boot import _ntff_profile_via_ctypes
            ah.set_axon_ntff_profile_hook(
                _ntff_profile_via_ctypes("/opt/axon/libaxon_pjrt.so"))
        from concourse import bass_utils as bu
        bu.upload_artifacts = lambda tmpdir: tmpdir   # no fish bucket here
        return True
    except Exception as e:                            # pragma: no cover
        print("trace setup failed:", e)
        return False


def kernel(**inputs):
    from concourse.bass_utils import run_bass_kernel_spmd

    assert int(np.asarray(inputs["k"])) == KNN
    if "nc" not in _prog_cache:
        _prog_cache["nc"] = _build_program()
    nc = _prog_cache["nc"]

    in_maps = _host_prep(inputs)
    trace = os.environ.get("KNN_TRACE") == "1" and _setup_trace()
    try:
        res = run_bass_kernel_spmd(nc, in_maps, list(range(NCORES)), trace=trace)
    except Exception:
        if not trace:
            raise
        res = run_bass_kernel_spmd(nc, in_maps, list(range(NCORES)), trace=False)
    if trace:
        _prog_cache["last_result"] = res

    out = np.concatenate([res.results[c]["out"] for c in range(NCORES)], 0)
    aux = np.concatenate([res.results[c]["aux"] for c in range(NCORES)], 0)
    _prog_cache["last_aux"] = aux
    out = _host_fix(out, aux, inputs)
    return out.astype(np.float32)
